# revision 3
# baseline (speedup 1.0000x reference)
"""GCN 2-layer (gcn_norm) SPMD Bass kernel for 8 TRN2 NeuronCores.

Strategy (node partition + edge partition by destination):
  - nodes sharded 6250/core; edges assigned to the core owning their dst.
  - layer math: out = dis * (sum_{e->v} dis[src]*h[src]) + dis^2*h_v + b
    with dis = deg^-1/2 (deg includes self-loop), h = x@W.
  - per layer: local projection -> scale by dis -> two half AllGathers
    (first/second half of each core's rows) into [25000,ch] tables in each
    core's HBM -> dma_gather rows for the core's edges (sorted by 128-node
    dst tile) -> indicator one-hot matmul scatter-adds each 128-edge chunk
    into the dst tile's PSUM accumulator -> epilogue.
  - int16 gather indices can only address 32767 rows, so nodes map into the
    two 25000-row tables: node v -> table (v%6250)//3125,
    row (v//6250)*3125 + (v%6250)%3125. Each tile's edges are grouped by
    table, each group padded to a multiple of 128 edges with index 0 /
    dst 255 (the indicator kills padding contributions).
  - dma_gather is capped at 1024 indices/instruction (SWDGE ring), so
    gathers are packed 8 chunks each and spread over 4 SWDGE queues.
"""

import numpy as np

N_NODES = 50000
N_EDGES = 800000
IN_CH = 128
HID = 64
OUT = 64
N_CORES = 8
PER_CORE = N_NODES // N_CORES          # 6250
N_TILES = (PER_CORE + 127) // 128      # 49
HALF_LOC = PER_CORE // 2               # 3125
TAB_ROWS = N_CORES * HALF_LOC          # 25000
PAD_DST = 255.0

_compiled_cache = {}


def _preprocess(edge_index: np.ndarray):
    """Host-side graph preprocessing -> per-core index/dst arrays + caps."""
    src = edge_index[0].astype(np.int64)
    dst = edge_index[1].astype(np.int64)

    deg = np.bincount(dst, minlength=N_NODES).astype(np.float64) + 1.0
    dis = (1.0 / np.sqrt(deg)).astype(np.float32)

    # table mapping: node v -> (half, row)
    src_core = src // PER_CORE
    src_r = src % PER_CORE
    half = (src_r >= HALF_LOC).astype(np.int64)
    tab_row = src_core * HALF_LOC + (src_r % HALF_LOC)

    core = dst // PER_CORE
    tile = (dst - core * PER_CORE) // 128
    order = np.lexsort((src, half, tile, core))
    row_s, dst_s = tab_row[order], dst[order]
    core_s, tile_s, half_s = core[order], tile[order], half[order]

    gid = (core_s * N_TILES + tile_s) * 2 + half_s
    counts = np.bincount(gid, minlength=N_CORES * N_TILES * 2).reshape(
        N_CORES, N_TILES, 2
    )
    cap128 = lambda x: max(128, int(-(-x // 128) * 128))
    cap_lo = cap128(counts[:, :, 0].max())
    cap_hi = cap128(counts[:, :, 1].max())
    c_lo, c_hi = cap_lo // 128, cap_hi // 128
    c_t = c_lo + c_hi

    starts = np.zeros(N_CORES * N_TILES * 2 + 1, dtype=np.int64)
    np.cumsum(counts.reshape(-1), out=starts[1:])

    per_core = []
    for c in range(N_CORES):
        idx_lo = np.zeros((N_TILES, cap_lo), dtype=np.int16)
        idx_hi = np.zeros((N_TILES, cap_hi), dtype=np.int16)
        dstc = np.full((N_TILES, c_t, 128), PAD_DST, dtype=np.float32)
        for t in range(N_TILES):
            g = (c * N_TILES + t) * 2
            n_lo = counts[c, t, 0]
            n_hi = counts[c, t, 1]
            s0 = starts[g]
            s1 = starts[g + 1]
            idx_lo[t, :n_lo] = row_s[s0:s0 + n_lo]
            idx_hi[t, :n_hi] = row_s[s1:s1 + n_hi]
            dloc = np.concatenate(
                [
                    dst_s[s0:s0 + n_lo] - c * PER_CORE - t * 128,
                    np.full(cap_lo - n_lo, PAD_DST),
                    dst_s[s1:s1 + n_hi] - c * PER_CORE - t * 128,
                    np.full(cap_hi - n_hi, PAD_DST),
                ]
            ).astype(np.float32)
            dstc[t] = dloc.reshape(c_t, 128)

        def wrap(a):  # [T, cap] -> [128, T*cap//16]
            w = a.reshape(N_TILES, -1, 16).transpose(2, 0, 1).reshape(16, -1)
            return np.tile(w, (8, 1)).copy()

        per_core.append(
            dict(
                idx_lo=wrap(idx_lo),
                idx_hi=wrap(idx_hi),
                dstc=dstc.transpose(2, 0, 1).reshape(128, -1).copy(),
            )
        )
    return dis, per_core, cap_lo, cap_hi


N_GTILES = (N_NODES + 127) // 128      # 391


def _build(cap_lo, cap_hi, do_gather=True, do_ind=True, do_mm=True):
    import concourse.bacc as bacc
    import concourse.mybir as mybir
    import concourse.tile as tile
    from concourse.bass import ds, ts

    c_lo, c_hi = cap_lo // 128, cap_hi // 128
    c_t = c_lo + c_hi
    f32 = mybir.dt.float32

    nc = bacc.Bacc("TRN2", target_bir_lowering=False, debug=False,
                   num_devices=N_CORES, dynamic_dma_scratch_size=65536,
                   num_swdge_queues=4)

    # I/O
    xT_d = nc.dram_tensor("xT", [IN_CH, PER_CORE], f32, kind="ExternalInput")
    w1_d = nc.dram_tensor("w1", [IN_CH, HID], f32, kind="ExternalInput")
    w2_d = nc.dram_tensor("w2", [HID, OUT], f32, kind="ExternalInput")
    b1_d = nc.dram_tensor("b1", [1, HID], f32, kind="ExternalInput")
    b2_d = nc.dram_tensor("b2", [1, OUT], f32, kind="ExternalInput")
    dis_d = nc.dram_tensor("dis_t", [128, N_TILES], f32, kind="ExternalInput")
    ixlo_d = nc.dram_tensor("idx_lo", [128, N_TILES * cap_lo // 16],
                            mybir.dt.int16, kind="ExternalInput")
    ixhi_d = nc.dram_tensor("idx_hi", [128, N_TILES * cap_hi // 16],
                            mybir.dt.int16, kind="ExternalInput")
    dstc_d = nc.dram_tensor("dstc", [128, N_TILES * c_t], f32,
                            kind="ExternalInput")
    out_d = nc.dram_tensor("out_local", [PER_CORE, OUT], f32,
                           kind="ExternalOutput")

    # internal DRAM: per-layer half bounces + half tables
    bnc = {}
    tab = {}
    for layer, ch in ((1, HID), (2, OUT)):
        for st in ("lo", "hi"):
            bnc[layer, st] = nc.dram_tensor(f"bounce{layer}{st}",
                                            [HALF_LOC, ch], f32,
                                            kind="Internal")
            tab[layer, st] = nc.dram_tensor(f"table{layer}{st}",
                                            [TAB_ROWS, ch], f32,
                                            kind="Internal",
                                            addr_space="Shared")

    iota_np = np.tile(np.arange(128, dtype=np.float32), (128, 1))
    ident_np = np.eye(128, dtype=np.float32)
    iota_d = nc.inline_tensor(iota_np, name="iota128")
    ident_d = nc.inline_tensor(ident_np, name="ident128")

    with tile.TileContext(nc) as tc:
        with (
            tc.tile_pool(name="const", bufs=1) as cpool,
            tc.tile_pool(name="state", bufs=1) as spool,
            tc.tile_pool(name="work", bufs=3) as wpool,
            tc.tile_pool(name="gath", bufs=12) as gpool,
            tc.tile_pool(name="ind", bufs=4) as ipool,
            tc.tile_pool(name="psA", bufs=2, space="PSUM") as psA,
            tc.tile_pool(name="psB", bufs=4, space="PSUM") as psB,
            tc.tile_pool(name="psT", bufs=2, space="PSUM") as psT,
        ):
            # ---- constants / inputs to SBUF ----
            iota_sb = cpool.tile([128, 128], f32, tag="iota")
            nc.sync.dma_start(iota_sb[:], iota_d[:])
            ident_sb = cpool.tile([128, 128], f32, tag="ident")
            nc.sync.dma_start(ident_sb[:], ident_d[:])
            w1_sb = cpool.tile([IN_CH, HID], f32, tag="w1")
            nc.sync.dma_start(w1_sb[:], w1_d[:])
            w2_sb = cpool.tile([HID, OUT], f32, tag="w2")
            nc.sync.dma_start(w2_sb[:], w2_d[:])
            dis_sb = cpool.tile([128, N_TILES], f32, tag="dis")
            nc.sync.dma_start(dis_sb[:], dis_d[:])
            b1_row = cpool.tile([1, HID], f32, tag="b1r")
            nc.sync.dma_start(b1_row[:], b1_d[:])
            b2_row = cpool.tile([1, OUT], f32, tag="b2r")
            nc.sync.dma_start(b2_row[:], b2_d[:])
            b1_bc = cpool.tile([128, HID], f32, tag="b1b")
            nc.gpsimd.partition_broadcast(b1_bc[:], b1_row[:])
            b2_bc = cpool.tile([128, OUT], f32, tag="b2b")
            nc.gpsimd.partition_broadcast(b2_bc[:], b2_row[:])
            ixlo_sb = cpool.tile([128, N_TILES * cap_lo // 16], mybir.dt.int16,
                                 tag="ixlo")
            nc.sync.dma_start(ixlo_sb[:], ixlo_d[:])
            ixhi_sb = cpool.tile([128, N_TILES * cap_hi // 16], mybir.dt.int16,
                                 tag="ixhi")
            nc.sync.dma_start(ixhi_sb[:], ixhi_d[:])
            dstc_sb = cpool.tile([128, N_TILES * c_t], f32, tag="dstc")
            nc.sync.dma_start(dstc_sb[:], dstc_d[:])

            # per-tile state tiles (fine-grained cross-phase deps)
            s1_t = [spool.tile([128, HID], f32, tag=f"s1_{t}", name=f"s1_{t}")
                    for t in range(N_TILES)]
            s2_t = [spool.tile([128, OUT], f32, tag=f"s2_{t}", name=f"s2_{t}")
                    for t in range(N_TILES)]
            h1_t = [spool.tile([128, HID], f32, tag=f"h1_{t}", name=f"h1_{t}")
                    for t in range(N_TILES)]
            nc.vector.memset(h1_t[N_TILES - 1][:], 0.0)

            def bounce_store(layer, t, nt, src_tile):
                """store [nt,ch] tile t rows into the lo/hi half bounces."""
                r0 = t * 128
                r1 = r0 + nt
                if r1 <= HALF_LOC:
                    nc.sync.dma_start(bnc[layer, "lo"][ds(r0, nt), :],
                                      src_tile[:nt, :])
                elif r0 >= HALF_LOC:
                    nc.sync.dma_start(bnc[layer, "hi"][ds(r0 - HALF_LOC, nt), :],
                                      src_tile[:nt, :])
                else:
                    n_a = HALF_LOC - r0
                    nc.sync.dma_start(bnc[layer, "lo"][ds(r0, n_a), :],
                                      src_tile[:n_a, :])
                    nc.sync.dma_start(bnc[layer, "hi"][ds(0, nt - n_a), :],
                                      src_tile[n_a:nt, :])

            def all_gather(layer, s):
                nc.gpsimd.collective_compute(
                    "AllGather", mybir.AluOpType.bypass,
                    replica_groups=[list(range(N_CORES))],
                    ins=[bnc[layer, s][:]], outs=[tab[layer, s][:]])

            def phase_a1(t, nt):
                """own x@W1 -> s1=dis^2*xW1+b1 (self-loop tile)."""
                xt = wpool.tile([IN_CH, 128], f32, tag="xto")
                nc.sync.dma_start(xt[:, :nt], xT_d[:, ds(t * 128, nt)])
                ps = psA.tile([128, HID], f32, tag="psa")
                nc.tensor.matmul(ps[:nt, :], xt[:, :nt], w1_sb[:],
                                 start=True, stop=True)
                hp = wpool.tile([128, HID], f32, tag="hp")
                dcol = dis_sb[:nt, t:t + 1]
                nc.scalar.mul(hp[:nt, :], ps[:nt, :], dcol)
                nc.vector.scalar_tensor_tensor(
                    s1_t[t][:nt, :], hp[:nt, :], dcol, b1_bc[:nt, :],
                    mybir.AluOpType.mult, mybir.AluOpType.add)
                bounce_store(1, t, nt, hp)

            def phase_a2(t, nt):
                """h1 tile -> transpose -> @W2 -> gp=dis*G -> s2, bounce."""
                pt = psT.tile([HID, 128], f32, tag="pst")
                nc.tensor.transpose(pt[:], h1_t[t][:], ident_sb[:])
                hT = wpool.tile([HID, 128], f32, tag="hT")
                nc.scalar.copy(hT[:], pt[:])
                ps = psA.tile([128, OUT], f32, tag="psa")
                nc.tensor.matmul(ps[:], hT[:], w2_sb[:], start=True, stop=True)
                gp = wpool.tile([128, OUT], f32, tag="gp")
                dcol = dis_sb[:nt, t:t + 1]
                nc.scalar.mul(gp[:nt, :], ps[:nt, :], dcol)
                nc.vector.scalar_tensor_tensor(
                    s2_t[t][:nt, :], gp[:nt, :], dcol, b2_bc[:nt, :],
                    mybir.AluOpType.mult, mybir.AluOpType.add)
                bounce_store(2, t, nt, gp)

            IDXG = 1024            # hard ucode cap per dma_gather
            CPG = IDXG // 128      # chunks per gather

            def phase_b(layer, ch, tile_done=None):
                """gather + indicator matmul scatter + epilogue."""
                gtiles = {"lo": {}, "hi": {}}
                qctr = [0]
                streams = {
                    "lo": (tab[layer, "lo"], ixlo_sb, N_TILES * c_lo),
                    "hi": (tab[layer, "hi"], ixhi_sb, N_TILES * c_hi),
                }

                def get_gather(stream, g):
                    if g in gtiles[stream]:
                        return gtiles[stream][g]
                    table_d, ix_sb, total = streams[stream]
                    n_ch = min(CPG, total - g * CPG)
                    tl = gpool.tile([128, CPG, ch], f32, tag="g" + stream)
                    if do_gather:
                        nc.gpsimd.dma_gather(
                            out_ap=tl[:, 0:n_ch, :],
                            in_ap=table_d[:],
                            idxs_ap=ix_sb[:, ds(g * IDXG // 16, n_ch * 8)],
                            num_idxs=n_ch * 128,
                            num_idxs_reg=n_ch * 128,
                            elem_size=ch,
                            queue_num=qctr[0] % 4,
                        )
                        qctr[0] += 1
                    else:
                        nc.vector.memset(tl[:], 0.0)
                    gtiles[stream][g] = tl
                    return tl

                for t in range(N_TILES):
                    nt = min(128, PER_CORE - t * 128)
                    ind = ipool.tile([128, c_t * 128], f32, tag="ind")
                    if do_ind:
                        for k in range(c_t):
                            col = t * c_t + k
                            if k % 2 == 1:
                                # ACT path: ind = relu(1 - |dst - iota|)
                                nc.scalar.activation(
                                    ind[:, ts(k, 128)], iota_sb[:],
                                    mybir.ActivationFunctionType.Abs,
                                    bias=dstc_sb[:, col:col + 1], scale=-1.0)
                                nc.scalar.activation(
                                    ind[:, ts(k, 128)], ind[:, ts(k, 128)],
                                    mybir.ActivationFunctionType.Relu,
                                    bias=1.0, scale=-1.0)
                            else:
                                nc.vector.tensor_scalar(
                                    ind[:, ts(k, 128)], iota_sb[:],
                                    dstc_sb[:, col:col + 1],
                                    None, mybir.AluOpType.is_equal)
                    else:
                        nc.vector.memset(ind[:], 0.0)
                    ps = psB.tile([128, ch], f32, tag="psb")
                    if do_mm:
                        for j in range(c_lo):
                            g, slot = divmod(t * c_lo + j, CPG)
                            tl = get_gather("lo", g)
                            nc.tensor.matmul(ps[:], ind[:, ts(j, 128)],
                                             tl[:, slot, :],
                                             start=(j == 0), stop=False)
                        for j in range(c_hi):
                            g, slot = divmod(t * c_hi + j, CPG)
                            tl = get_gather("hi", g)
                            nc.tensor.matmul(ps[:], ind[:, ts(c_lo + j, 128)],
                                             tl[:, slot, :],
                                             start=False, stop=(j == c_hi - 1))
                    else:
                        tl = get_gather("lo", (t * c_lo) // CPG)
                        nc.tensor.matmul(ps[:], ind[:, ts(0, 128)],
                                         tl[:, (t * c_lo) % CPG, :],
                                         start=True, stop=True)
                    dcol = dis_sb[:nt, t:t + 1]
                    if layer == 1:
                        nc.vector.scalar_tensor_tensor(
                            h1_t[t][:nt, :], ps[:nt, :], dcol, s1_t[t][:nt, :],
                            mybir.AluOpType.mult, mybir.AluOpType.add)
                        nc.scalar.activation(
                            h1_t[t][:nt, :], h1_t[t][:nt, :],
                            mybir.ActivationFunctionType.Relu)
                    else:
                        ot = wpool.tile([128, OUT], f32, tag="ot")
                        nc.vector.scalar_tensor_tensor(
                            ot[:nt, :], ps[:nt, :], dcol, s2_t[t][:nt, :],
                            mybir.AluOpType.mult, mybir.AluOpType.add)
                        nc.sync.dma_start(out_d[ds(t * 128, nt), :],
                                          ot[:nt, :])
                    if tile_done is not None:
                        tile_done(t, nt)

            # ---------- layer 1 ----------
            for t in range(N_TILES):
                phase_a1(t, min(128, PER_CORE - t * 128))
            all_gather(1, "lo")
            all_gather(1, "hi")

            # layer-2 phase A runs per-tile as layer-1 phase B finishes tiles
            def l1_done(t, nt):
                phase_a2(t, nt)
                if t == 40:
                    all_gather(2, "lo")
                elif t == N_TILES - 1:
                    all_gather(2, "hi")

            phase_b(1, HID, tile_done=l1_done)

            # ---------- layer 2 ----------
            phase_b(2, OUT)

    nc.compile()
    return nc


def _make_in_maps(x, W1, b1, W2, b2, dis, per_core):
    in_maps = []
    for c in range(N_CORES):
        dis_c = np.zeros(N_TILES * 128, dtype=np.float32)
        dis_c[:PER_CORE] = dis[c * PER_CORE:(c + 1) * PER_CORE]
        in_maps.append(
            {
                "xT": np.ascontiguousarray(
                    x[c * PER_CORE:(c + 1) * PER_CORE].T),
                "w1": np.ascontiguousarray(W1),
                "w2": np.ascontiguousarray(W2),
                "b1": np.ascontiguousarray(b1.reshape(1, -1)),
                "b2": np.ascontiguousarray(b2.reshape(1, -1)),
                "dis_t": np.ascontiguousarray(
                    dis_c.reshape(N_TILES, 128).T),
                "idx_lo": per_core[c]["idx_lo"],
                "idx_hi": per_core[c]["idx_hi"],
                "dstc": per_core[c]["dstc"],
            }
        )
    return in_maps


def run(x, edge_index, W1, b1, W2, b2, trace=False, tmpdir=None):
    from concourse.bass_utils import run_bass_kernel_spmd

    x = np.asarray(x, dtype=np.float32)
    edge_index = np.asarray(edge_index)
    W1 = np.asarray(W1, dtype=np.float32)
    b1 = np.asarray(b1, dtype=np.float32)
    W2 = np.asarray(W2, dtype=np.float32)
    b2 = np.asarray(b2, dtype=np.float32)

    dis, per_core, cap_lo, cap_hi = _preprocess(edge_index)
    key = (cap_lo, cap_hi)
    if key not in _compiled_cache:
        _compiled_cache[key] = _build(cap_lo, cap_hi)
    nc = _compiled_cache[key]
    in_maps = _make_in_maps(x, W1, b1, W2, b2, dis, per_core)
    res = run_bass_kernel_spmd(nc, in_maps, core_ids=list(range(N_CORES)),
                               trace=trace, tmpdir=tmpdir)
    out = np.concatenate([res.results[c]["out_local"] for c in range(N_CORES)],
                         axis=0)
    return out, res


def kernel(x, edge_index, W1, b1, W2, b2):
    out, _ = run(x, edge_index, W1, b1, W2, b2, trace=False)
    return out



# revision 4
# speedup vs baseline: 1.0058x; 1.0058x over previous
"""GCN 2-layer SPMD Bass kernel v2b for 8 TRN2 NeuronCores.

Design:
  - No layer-1 collective: full x replicated; each core projects the whole
    table1 = dis*(x@W1) locally in 8-tile groups (big contiguous DMAs,
    full 256B bf16-padded rows so writes coalesce).
  - Layer-2: per-tile epilogue computes gp2 = dis*(h1@W2); rows bounce to
    two chunk tables AllGather'd as soon as their rows exist (issued from
    the Scalar engine so GpSimd keeps streaming gathers).
  - Tables are [rows, 128] bf16 (64 payload + 64 pad = 256B gather elems).
    Table splits are tile-aligned: global 24576 (192 tiles), local 3072.
  - Scatter: per dst tile, one stride-0-broadcast tensor_tensor is_equal
    builds the whole bf16 indicator; PE does bf16 one-hot matmuls.
  - Gathers: 1024-idx dma_gather, 4 SWDGE queues round-robin, 96KB
    descriptor scratch, 10-deep tile pool; per-tile chunk caps.
"""

import numpy as np

N_NODES = 50000
N_EDGES = 800000
IN_CH = 128
HID = 64
OUT = 64
N_CORES = 8
PER_CORE = N_NODES // N_CORES          # 6250
N_TILES = (PER_CORE + 127) // 128      # 49
N_GTILES = (N_NODES + 127) // 128      # 391
HALF_GLOB = 24576                      # layer-1 lo/hi split (192 tiles)
HI_GLOB = N_NODES - HALF_GLOB          # 25424
HI_GLOB_PAD = 25472                    # hi table padded to full perm groups
HALF_LOC = 3072                        # layer-2 chunk split (24 tiles)
HI_LOC = PER_CORE - HALF_LOC           # 3178
CHUNK0_ROWS = N_CORES * HALF_LOC       # 24576
CHUNK1_ROWS = N_CORES * HI_LOC         # 25424
PAD_DST = 255.0
GRP = 16                               # proj tiles per DMA group

_compiled_cache = {}


def _pack_idx_flat(parts):
    """list of [cap_t] int16 -> [128, sum(cap)/16] wrapped."""
    a = np.concatenate(parts)
    w = a.reshape(-1, 16).T
    return np.tile(w, (8, 1)).copy()


def _preprocess(edge_index: np.ndarray):
    src = edge_index[0].astype(np.int64)
    dst = edge_index[1].astype(np.int64)

    deg = np.bincount(dst, minlength=N_NODES).astype(np.float64) + 1.0
    dis = (1.0 / np.sqrt(deg)).astype(np.float32)

    core = dst // PER_CORE
    tile = (dst - core * PER_CORE) // 128

    half1 = (src >= HALF_GLOB).astype(np.int64)
    # table1 rows are permuted partition-major within 16-tile groups:
    # node v -> group g=v//2048, p=v%128, k=(v%2048)//128, row g*2048+p*gn+k
    def perm_row(v):
        g = v // 2048
        r = v % 2048
        k = r // 128
        p = r % 128
        gmax = (NV_HALF[(v >= HALF_GLOB).astype(np.int64)]
                if False else None)
        return g, k, p
    v1 = np.where(half1 == 0, src, src - HALF_GLOB)
    g1 = v1 // 2048
    k1 = (v1 % 2048) // 128
    p1 = v1 % 128
    # tiles per group: lo always 16; hi last group (g=12) has 7
    hi_last_g = HI_GLOB // 2048            # 12
    gn1 = np.where(half1 == 1, np.where(g1 == hi_last_g, 7, 16), 16)
    row1 = g1 * 2048 + p1 * gn1 + k1
    src_core = src // PER_CORE
    src_loc = src % PER_CORE
    half2 = (src_loc >= HALF_LOC).astype(np.int64)
    row2 = np.where(half2 == 0, src_core * HALF_LOC + src_loc,
                    src_core * HI_LOC + (src_loc - HALF_LOC))

    def group(halfx, rowx):
        order = np.lexsort((rowx, halfx, tile, core))
        gid = (core[order] * N_TILES + tile[order]) * 2 + halfx[order]
        counts = np.bincount(gid, minlength=N_CORES * N_TILES * 2).reshape(
            N_CORES, N_TILES, 2)
        # per-tile caps (max over cores), multiple of 128
        caps = np.maximum(128, ((counts.max(axis=0) + 127) // 128) * 128)
        starts = np.zeros(N_CORES * N_TILES * 2 + 1, dtype=np.int64)
        np.cumsum(counts.reshape(-1), out=starts[1:])
        return order, counts, starts, caps  # caps: [N_TILES, 2]

    o1, cnt1, st1, caps1 = group(half1, row1)
    o2, cnt2, st2, caps2 = group(half2, row2)

    per_core = []
    for c in range(N_CORES):
        def build(order, counts, starts, caps, rowx):
            row_s = rowx[order]
            dst_s = dst[order]
            lo_parts, hi_parts, dv_parts = [], [], []
            for t in range(N_TILES):
                g = (c * N_TILES + t) * 2
                n_lo, n_hi = counts[c, t, 0], counts[c, t, 1]
                cap_lo, cap_hi = int(caps[t, 0]), int(caps[t, 1])
                s0, s1 = starts[g], starts[g + 1]
                ilo = np.zeros(cap_lo, dtype=np.int16)
                ilo[:n_lo] = row_s[s0:s0 + n_lo]
                ihi = np.zeros(cap_hi, dtype=np.int16)
                ihi[:n_hi] = row_s[s1:s1 + n_hi]
                lo_parts.append(ilo)
                hi_parts.append(ihi)
                d = np.concatenate([
                    dst_s[s0:s0 + n_lo] - c * PER_CORE - t * 128,
                    np.full(cap_lo - n_lo, PAD_DST),
                    dst_s[s1:s1 + n_hi] - c * PER_CORE - t * 128,
                    np.full(cap_hi - n_hi, PAD_DST),
                ]).astype(np.float32)
                dv_parts.append(d.reshape(-1, 128).T)  # [128, ct_t]
            dstv = np.concatenate(dv_parts, axis=1)    # [128, sum ct]
            return (_pack_idx_flat(lo_parts), _pack_idx_flat(hi_parts), dstv)

        i1lo, i1hi, dv1 = build(o1, cnt1, st1, caps1, row1)
        i2lo, i2hi, dv2 = build(o2, cnt2, st2, caps2, row2)
        per_core.append(dict(idx1lo=i1lo, idx1hi=i1hi, dstv1=dv1,
                             idx2lo=i2lo, idx2hi=i2hi, dstv2=dv2))
    caps_key = (tuple(caps1.reshape(-1).tolist()),
                tuple(caps2.reshape(-1).tolist()))
    return dis, per_core, (caps1, caps2), caps_key


def _build(caps1, caps2):
    import concourse.bacc as bacc
    import concourse.mybir as mybir
    import concourse.tile as tile
    from concourse.bass import AP, ds

    clo1 = [int(caps1[t, 0]) // 128 for t in range(N_TILES)]
    chi1 = [int(caps1[t, 1]) // 128 for t in range(N_TILES)]
    clo2 = [int(caps2[t, 0]) // 128 for t in range(N_TILES)]
    chi2 = [int(caps2[t, 1]) // 128 for t in range(N_TILES)]
    ct1 = [a + b for a, b in zip(clo1, chi1)]
    ct2 = [a + b for a, b in zip(clo2, chi2)]
    f32 = mybir.dt.float32
    bf16 = mybir.dt.bfloat16

    nc = bacc.Bacc("TRN2", target_bir_lowering=False, debug=False,
                   num_devices=N_CORES, dynamic_dma_scratch_size=98304,
                   num_swdge_queues=4)

    # ---- I/O ----
    NXG = (N_GTILES + GRP - 1) // GRP
    NOG = (N_TILES + GRP - 1) // GRP
    xT_d = nc.dram_tensor("xTg", [NXG * IN_CH, GRP * 128], bf16,
                          kind="ExternalInput")
    xTo_d = nc.dram_tensor("xTog", [NOG * IN_CH, GRP * 128], bf16,
                           kind="ExternalInput")
    w1_d = nc.dram_tensor("w1b", [IN_CH, HID], bf16, kind="ExternalInput")
    w2_d = nc.dram_tensor("w2b", [HID, OUT], bf16, kind="ExternalInput")
    b1_d = nc.dram_tensor("b1", [1, HID], f32, kind="ExternalInput")
    b2_d = nc.dram_tensor("b2", [1, OUT], f32, kind="ExternalInput")
    disg_d = nc.dram_tensor("disg", [128, N_GTILES], f32, kind="ExternalInput")
    disl_d = nc.dram_tensor("disl", [128, N_TILES], f32, kind="ExternalInput")
    disq_d = nc.dram_tensor("dislq", [128, N_TILES], f32, kind="ExternalInput")
    n1lo, n1hi = sum(clo1) * 8, sum(chi1) * 8
    n2lo, n2hi = sum(clo2) * 8, sum(chi2) * 8
    ix1lo_d = nc.dram_tensor("idx1lo", [128, n1lo], mybir.dt.int16,
                             kind="ExternalInput")
    ix1hi_d = nc.dram_tensor("idx1hi", [128, n1hi], mybir.dt.int16,
                             kind="ExternalInput")
    ix2lo_d = nc.dram_tensor("idx2lo", [128, n2lo], mybir.dt.int16,
                             kind="ExternalInput")
    ix2hi_d = nc.dram_tensor("idx2hi", [128, n2hi], mybir.dt.int16,
                             kind="ExternalInput")
    dstv1_d = nc.dram_tensor("dstv1", [128, sum(ct1)], bf16,
                             kind="ExternalInput")
    dstv2_d = nc.dram_tensor("dstv2", [128, sum(ct2)], bf16,
                             kind="ExternalInput")
    iota_d = nc.dram_tensor("iotab", [128, 128], bf16, kind="ExternalInput")
    ident_d = nc.dram_tensor("identb", [128, 128], bf16, kind="ExternalInput")
    out_d = nc.dram_tensor("out_local", [PER_CORE, OUT], f32,
                           kind="ExternalOutput")

    # ---- internal DRAM ----
    tab1 = [nc.dram_tensor("tab1lo", [HALF_GLOB, 128], bf16, kind="Internal"),
            nc.dram_tensor("tab1hi", [HI_GLOB_PAD, 128], bf16,
                           kind="Internal")]
    bnc2 = [nc.dram_tensor("bnc2lo", [HALF_LOC, 128], bf16, kind="Internal"),
            nc.dram_tensor("bnc2hi", [HI_LOC, 128], bf16, kind="Internal")]
    tab2 = [nc.dram_tensor("tab2lo", [CHUNK0_ROWS, 128], bf16,
                           kind="Internal", addr_space="Shared"),
            nc.dram_tensor("tab2hi", [CHUNK1_ROWS, 128], bf16,
                           kind="Internal", addr_space="Shared")]

    with tile.TileContext(nc) as tc:
        with (
            tc.tile_pool(name="const", bufs=1) as cpool,
            tc.tile_pool(name="state", bufs=1) as spool,
            tc.tile_pool(name="proj", bufs=3) as ppool,
            tc.tile_pool(name="proj2", bufs=2) as gpool2,
            tc.tile_pool(name="work", bufs=2) as wpool,
            tc.tile_pool(name="gath", bufs=10) as gpool,
            tc.tile_pool(name="ixp", bufs=1) as ixpool,
            tc.tile_pool(name="ind", bufs=2) as ipool,
            tc.tile_pool(name="psA", bufs=2, space="PSUM") as psA,
            tc.tile_pool(name="psB", bufs=3, space="PSUM") as psB,
            tc.tile_pool(name="psT", bufs=1, space="PSUM") as psT,
        ):
            # ---- constants ----
            iota_sb = cpool.tile([128, 128], bf16, tag="iota")
            nc.sync.dma_start(iota_sb[:], iota_d[:])
            ident_sb = cpool.tile([128, 128], bf16, tag="ident")
            nc.sync.dma_start(ident_sb[:], ident_d[:])
            w1_sb = cpool.tile([IN_CH, HID], bf16, tag="w1")
            nc.sync.dma_start(w1_sb[:], w1_d[:])
            w2_sb = cpool.tile([HID, OUT], bf16, tag="w2")
            nc.sync.dma_start(w2_sb[:], w2_d[:])
            disg_sb = cpool.tile([128, N_GTILES], f32, tag="disg")
            nc.sync.dma_start(disg_sb[:], disg_d[:])
            disl_sb = cpool.tile([128, N_TILES], f32, tag="disl")
            nc.sync.dma_start(disl_sb[:], disl_d[:])
            disq_sb = cpool.tile([128, N_TILES], f32, tag="dislq")
            nc.sync.dma_start(disq_sb[:], disq_d[:])
            b1_row = cpool.tile([1, HID], f32, tag="b1r")
            nc.sync.dma_start(b1_row[:], b1_d[:])
            b2_row = cpool.tile([1, OUT], f32, tag="b2r")
            nc.sync.dma_start(b2_row[:], b2_d[:])
            b1_bc = cpool.tile([128, HID], f32, tag="b1b")
            nc.gpsimd.partition_broadcast(b1_bc[:], b1_row[:])
            b2_bc = cpool.tile([128, OUT], f32, tag="b2b")
            nc.gpsimd.partition_broadcast(b2_bc[:], b2_row[:])
            def load_ix(layer, dlo, dhi, nlo, nhi):
                tlo = ixpool.tile([128, nlo], mybir.dt.int16, tag="ixlo",
                                  name=f"ixlo{layer}")
                nc.sync.dma_start(tlo[:], dlo[:])
                thi = ixpool.tile([128, nhi], mybir.dt.int16, tag="ixhi",
                                  name=f"ixhi{layer}")
                nc.sync.dma_start(thi[:], dhi[:])
                return tlo, thi
            dstv1_sb = cpool.tile([128, sum(ct1)], bf16, tag="dstv1")
            nc.sync.dma_start(dstv1_sb[:], dstv1_d[:])
            dstv2_sb = cpool.tile([128, sum(ct2)], bf16, tag="dstv2")
            nc.sync.dma_start(dstv2_sb[:], dstv2_d[:])

            s1_all = spool.tile([128, N_TILES, HID], f32, tag="s1a",
                                name="s1a")
            s1_t = [s1_all[:, t, :] for t in range(N_TILES)]
            s2_t = [spool.tile([128, OUT], f32, tag=f"s2_{t}",
                               name=f"s2_{t}") for t in range(N_TILES)]

            qctr = [0]

            def next_q():
                q = qctr[0] % 4
                qctr[0] += 1
                return q

            # ---------- layer-1 projection: grouped tiles ----------
            def proj1():
                for g0 in range(0, N_GTILES, GRP):
                    gn = min(GRP, N_GTILES - g0)
                    ncols = min(gn * 128, N_NODES - g0 * 128)
                    xt = ppool.tile([IN_CH, GRP * 128], bf16, tag="xt")
                    nc.sync.dma_start(
                        xt[:, :ncols],
                        xT_d[ds((g0 // GRP) * IN_CH, IN_CH), 0:ncols])
                    gtt = gpool2.tile([128, GRP, 128], bf16, tag="gtt")
                    for q0 in range(0, gn, 8):
                        qn = min(8, gn - q0)
                        ps = psA.tile([128, 8, HID], f32, tag="psa",
                                      name=f"psa_{g0}_{q0}")
                        for k in range(q0, q0 + qn):
                            gt = g0 + k
                            ng = min(128, N_NODES - gt * 128)
                            nc.tensor.matmul(ps[:ng, k - q0, :],
                                             xt[:, k * 128:k * 128 + ng],
                                             w1_sb[:], start=True, stop=True)
                        # one evac op per octet: gtt[:, k, 0:HID] = ps * dis
                        dc = disg_sb[:, g0 + q0:g0 + q0 + qn]
                        dce = AP(dc.tensor, dc.offset,
                                 [dc.ap[0], (1, qn), (0, HID)])
                        dst = gtt[:, q0:q0 + qn, 0:HID]
                        nc.vector.tensor_tensor(dst, ps[:, 0:qn, :], dce,
                                                mybir.AluOpType.mult)
                    # one write per group; table rows are partition-major
                    # within the group (row = grpbase + p*gn + k), so each
                    # partition's SBUF bytes are one contiguous DRAM run
                    r0 = g0 * 128
                    tb = tab1[0] if r0 < HALF_GLOB else tab1[1]
                    off = (r0 if r0 < HALF_GLOB else r0 - HALF_GLOB) * 128
                    base = tb[:]
                    dst = AP(base.tensor, off,
                             [(gn * 128, 128), (128, gn), (1, 128)])
                    nc.scalar.dma_start(dst, gtt[:, 0:gn, :])

            # ---------- self terms ----------
            def self_terms1():
                for t0 in range(0, N_TILES, GRP):
                    gn = min(GRP, N_TILES - t0)
                    ncols = min(gn * 128, PER_CORE - t0 * 128)
                    xt = ppool.tile([IN_CH, GRP * 128], bf16, tag="xt")
                    nc.sync.dma_start(
                        xt[:, :ncols],
                        xTo_d[ds((t0 // GRP) * IN_CH, IN_CH), 0:ncols])
                    for q0 in range(0, gn, 8):
                        qn = min(8, gn - q0)
                        ps = psA.tile([128, 8, HID], f32, tag="psa",
                                      name=f"psl_{t0}_{q0}")
                        for k in range(q0, q0 + qn):
                            t = t0 + k
                            nt = min(128, PER_CORE - t * 128)
                            nc.tensor.matmul(ps[:nt, k - q0, :],
                                             xt[:, k * 128:k * 128 + nt],
                                             w1_sb[:], start=True, stop=True)
                        dq = disq_sb[:, t0 + q0:t0 + q0 + qn]
                        dqe = AP(dq.tensor, dq.offset,
                                 [dq.ap[0], (1, qn), (0, HID)])
                        tmp = wpool.tile([128, 8, HID], f32, tag="stmp")
                        nc.vector.tensor_tensor(tmp[:, 0:qn, :],
                                                ps[:, 0:qn, :], dqe,
                                                mybir.AluOpType.mult)
                        b1a = b1_bc[:, :]
                        b1e = AP(b1a.tensor, b1a.offset,
                                 [b1a.ap[0], (0, qn), (1, HID)])
                        nc.vector.tensor_tensor(
                            s1_all[:, t0 + q0:t0 + q0 + qn, :],
                            tmp[:, 0:qn, :], b1e, mybir.AluOpType.add)

            IDXG = 1024
            CPG = IDXG // 128

            def phase_b(tables, ixlo, ixhi, clo, chi, dstv_sb, ch, tile_done):
                # per-stream chunk offsets
                cum_lo = np.concatenate([[0], np.cumsum(clo)]).astype(int)
                cum_hi = np.concatenate([[0], np.cumsum(chi)]).astype(int)
                cum_ct = np.concatenate(
                    [[0], np.cumsum([a + b for a, b in zip(clo, chi)])]
                ).astype(int)
                gtiles = {0: {}, 1: {}}
                streams = {0: (tables[0], ixlo, int(cum_lo[-1])),
                           1: (tables[1], ixhi, int(cum_hi[-1]))}

                def get_gather(stream, g):
                    if g in gtiles[stream]:
                        return gtiles[stream][g]
                    table_d, ix, total = streams[stream]
                    n_ch = min(CPG, total - g * CPG)
                    tl = gpool.tile([128, CPG, 128], bf16, tag=f"g{stream}")
                    nc.gpsimd.dma_gather(
                        out_ap=tl[:, 0:n_ch, :],
                        in_ap=table_d[:],
                        idxs_ap=ix[:, ds(g * IDXG // 16, n_ch * 8)],
                        num_idxs=n_ch * 128,
                        num_idxs_reg=n_ch * 128,
                        elem_size=128,
                        queue_num=next_q(),
                    )
                    gtiles[stream][g] = tl
                    return tl

                for t in range(N_TILES):
                    nt = min(128, PER_CORE - t * 128)
                    ct = int(cum_ct[t + 1] - cum_ct[t])
                    ind = ipool.tile([128, ct, 128], bf16, tag="ind")
                    iota_ap = iota_sb[:]
                    iota_rep = AP(iota_ap.tensor, iota_ap.offset,
                                  [iota_ap.ap[0], (0, ct), (1, 128)])
                    dcols = dstv_sb[:, int(cum_ct[t]):int(cum_ct[t + 1])]
                    dstb = AP(dcols.tensor, dcols.offset,
                              [dcols.ap[0], (1, ct), (0, 128)])
                    nc.vector.tensor_tensor(ind[:], iota_rep, dstb,
                                            mybir.AluOpType.is_equal)
                    ps = psB.tile([128, ch], f32, tag="psb")
                    nlo, nhi = int(clo[t]), int(chi[t])
                    for j in range(nlo):
                        gidx = int(cum_lo[t]) + j
                        g, slot = divmod(gidx, CPG)
                        tl = get_gather(0, g)
                        nc.tensor.matmul(ps[:], ind[:, j, :],
                                         tl[:, slot, 0:ch],
                                         start=(j == 0), stop=False)
                    for j in range(nhi):
                        gidx = int(cum_hi[t]) + j
                        g, slot = divmod(gidx, CPG)
                        tl = get_gather(1, g)
                        nc.tensor.matmul(ps[:], ind[:, nlo + j, :],
                                         tl[:, slot, 0:ch],
                                         start=False, stop=(j == nhi - 1))
                    tile_done(t, nt, ps)

            # ---------- layer-1 epilogue + layer-2 prep ----------
            def l1_done(t, nt, ps):
                dcol = disl_sb[:nt, t:t + 1]
                h1 = wpool.tile([128, HID], bf16, tag="h1")
                hf = wpool.tile([128, HID], f32, tag="hf")
                nc.vector.scalar_tensor_tensor(
                    hf[:nt, :], ps[:nt, :], dcol, s1_t[t][:nt, :],
                    mybir.AluOpType.mult, mybir.AluOpType.add)
                if nt < 128:
                    nc.vector.memset(h1[:], 0.0)
                nc.scalar.activation(h1[:nt, :], hf[:nt, :],
                                     mybir.ActivationFunctionType.Relu)
                pt = psT.tile([HID, 128], bf16, tag="pst")
                nc.tensor.transpose(pt[:], h1[:], ident_sb[:])
                hT = wpool.tile([HID, 128], bf16, tag="hT")
                nc.scalar.copy(hT[:], pt[:])
                ps2 = psA.tile([128, OUT], f32, tag="ps2", name=f"ps2_{t}")
                nc.tensor.matmul(ps2[:], hT[:], w2_sb[:], start=True,
                                 stop=True)
                gp = wpool.tile([128, 128], bf16, tag="gp")
                nc.scalar.mul(gp[:nt, 0:OUT], ps2[:nt, :], dcol)
                dq = disq_sb[:nt, t:t + 1]
                nc.vector.scalar_tensor_tensor(
                    s2_t[t][:nt, :], ps2[:nt, :], dq, b2_bc[:nt, :],
                    mybir.AluOpType.mult, mybir.AluOpType.add)
                # bounce gp rows (full 256B rows; tile-aligned split)
                r0 = t * 128
                if r0 + nt <= HALF_LOC:
                    nc.scalar.dma_start(bnc2[0][ds(r0, nt), :], gp[:nt, :])
                else:
                    nc.scalar.dma_start(bnc2[1][ds(r0 - HALF_LOC, nt), :],
                                      gp[:nt, :])
                if t == HALF_LOC // 128 - 1:
                    nc.gpsimd.collective_compute(
                        "AllGather", mybir.AluOpType.bypass,
                        replica_groups=[list(range(N_CORES))],
                        ins=[bnc2[0][:]], outs=[tab2[0][:]])
                elif t == N_TILES - 1:
                    nc.gpsimd.collective_compute(
                        "AllGather", mybir.AluOpType.bypass,
                        replica_groups=[list(range(N_CORES))],
                        ins=[bnc2[1][:]], outs=[tab2[1][:]])

            def l2_done(t, nt, ps):
                dcol = disl_sb[:nt, t:t + 1]
                ot = wpool.tile([128, OUT], f32, tag="ot")
                nc.vector.scalar_tensor_tensor(
                    ot[:nt, :], ps[:nt, :], dcol, s2_t[t][:nt, :],
                    mybir.AluOpType.mult, mybir.AluOpType.add)
                nc.scalar.dma_start(out_d[ds(t * 128, nt), :], ot[:nt, :])

            # ---------- schedule ----------
            ix1l, ix1h = load_ix(1, ix1lo_d, ix1hi_d, n1lo, n1hi)
            proj1()
            self_terms1()
            phase_b(tab1, ix1l, ix1h, clo1, chi1, dstv1_sb, HID, l1_done)
            ix2l, ix2h = load_ix(2, ix2lo_d, ix2hi_d, n2lo, n2hi)
            phase_b(tab2, ix2l, ix2h, clo2, chi2, dstv2_sb, OUT, l2_done)

    nc.compile()
    return nc


def _make_in_maps(x, W1, b1, W2, b2, dis, per_core):
    import ml_dtypes
    bf = ml_dtypes.bfloat16
    xT = np.ascontiguousarray(x.T.astype(bf))

    def group_major(xt_cols):  # [128, C] -> [ceil(C/2048)*128, 2048]
        C = xt_cols.shape[1]
        ngrp = (C + GRP * 128 - 1) // (GRP * 128)
        out = np.zeros((ngrp * IN_CH, GRP * 128), dtype=xt_cols.dtype)
        for g in range(ngrp):
            c0 = g * GRP * 128
            w = min(GRP * 128, C - c0)
            out[g * IN_CH:(g + 1) * IN_CH, :w] = xt_cols[:, c0:c0 + w]
        return out
    disg = np.zeros(N_GTILES * 128, dtype=np.float32)
    disg[:N_NODES] = dis
    disg = np.ascontiguousarray(disg.reshape(N_GTILES, 128).T)
    iota = np.tile(np.arange(128, dtype=np.float32), (128, 1)).astype(bf)
    ident = np.eye(128, dtype=np.float32).astype(bf)
    w1b = np.ascontiguousarray(W1.astype(bf))
    w2b = np.ascontiguousarray(W2.astype(bf))
    xTg = group_major(xT)
    in_maps = []
    for c in range(N_CORES):
        disl = np.zeros(N_TILES * 128, dtype=np.float32)
        disl[:PER_CORE] = dis[c * PER_CORE:(c + 1) * PER_CORE]
        pc = per_core[c]
        in_maps.append({
            "xTg": xTg,
            "xTog": group_major(xT[:, c * PER_CORE:(c + 1) * PER_CORE]),
            "w1b": w1b,
            "w2b": w2b,
            "b1": np.ascontiguousarray(b1.reshape(1, -1)),
            "b2": np.ascontiguousarray(b2.reshape(1, -1)),
            "disg": disg,
            "disl": np.ascontiguousarray(disl.reshape(N_TILES, 128).T),
            "dislq": np.ascontiguousarray((disl * disl).reshape(N_TILES, 128).T),
            "idx1lo": pc["idx1lo"], "idx1hi": pc["idx1hi"],
            "idx2lo": pc["idx2lo"], "idx2hi": pc["idx2hi"],
            "dstv1": np.ascontiguousarray(pc["dstv1"].astype(bf)),
            "dstv2": np.ascontiguousarray(pc["dstv2"].astype(bf)),
            "iotab": iota,
            "identb": ident,
        })
    return in_maps


def run(x, edge_index, W1, b1, W2, b2, trace=False, tmpdir=None):
    from concourse.bass_utils import run_bass_kernel_spmd

    x = np.asarray(x, dtype=np.float32)
    edge_index = np.asarray(edge_index)
    W1 = np.asarray(W1, dtype=np.float32)
    b1 = np.asarray(b1, dtype=np.float32)
    W2 = np.asarray(W2, dtype=np.float32)
    b2 = np.asarray(b2, dtype=np.float32)

    dis, per_core, (caps1, caps2), key = _preprocess(edge_index)
    if key not in _compiled_cache:
        _compiled_cache[key] = _build(caps1, caps2)
    nc = _compiled_cache[key]
    in_maps = _make_in_maps(x, W1, b1, W2, b2, dis, per_core)
    res = run_bass_kernel_spmd(nc, in_maps, core_ids=list(range(N_CORES)),
                               trace=trace, tmpdir=tmpdir)
    out = np.concatenate([res.results[c]["out_local"] for c in range(N_CORES)],
                         axis=0)
    return out, res


def kernel(x, edge_index, W1, b1, W2, b2):
    out, _ = run(x, edge_index, W1, b1, W2, b2, trace=False)
    return out
